# revision 19
# baseline (speedup 1.0000x reference)
"""Trainium2 Bass kernel for nn_BoxMultiHeadedAttention_81312320848177.

Self-contained: kernel(**inputs) takes FULL inputs, shards batch over 8
NeuronCores (2 batches/core), runs a Tile/Bass kernel per core, gathers.

Per-core algorithm (B_local=2, N=256, D=1024, H=8, DK=128):
- QKV/O projections in bf16 (host-converted weights, 1/sqrt(DK) folded into
  Wq), fp32 PSUM accumulate; issue-interleaved with the geo loop so the PE
  fills dependency gaps.
- Box relational embedding phases in an f-major packed partition layout
  (p = f*16 + ii, 8 freqs x 16 i's); per-group operand expansion is done by
  an SBUF->SBUF broadcast DMA (stride-0 partition replication) instead of
  PE selection-matmuls, carrying full fp32 precision (no hi/lo split).
- |delta|/clip fused into one custom DVE op (ABSCLIP); all four ln() calls
  fused into a single wide ACT Ln (also pins act-table order: Ln before Sin).
- Range reduction r = u - round(u) via custom DVE op (FRAC); sin/cos of the
  pairwise phases via one ACT Sin pass (cos via +0.25 phase shift).
- V33 (separable features) sin via a second custom DVE op (SINPOLY,
  degree-7 odd minimax of sin(2*pi*r)) so ACT never needs the trig table
  before the Ln block completes.
- geo = relu(emb @ Wg^T + bg) via block-diagonal packed matmuls (f-major),
  separable dw/dh folded into a rank-32 bilinear form => one extra k=33
  matmul per i-group; relu on the Pool engine.
- Softmax without max-subtraction; transposed orientation (j on partitions);
  1/denom applied post-attnV. Exp is gated behind the last geo tile by a
  zero-weight guard matmul so the act-table never thrashes Sin<->Exp.
"""
import sys
sys.path.insert(0, '/opt/trn_rl_repo')

import numpy as np
from contextlib import ExitStack

B, N, D, H, DK = 16, 256, 1024, 8, 128
BL = 2                 # batches per core
NCORES = 8
WAVE_LEN = 1000.0
C_ROUND = float(1.5 * 2**23)
TWO_PI = float(2 * np.pi)
INV_SQRT_DK = float(1.0 / np.sqrt(DK))

# degree-7 odd minimax of sin(2*pi*r) on [-0.5, 0.5]; max abs err 2.5e-4
SC0, SC1, SC2, SC3 = 6.27863516, -41.09372018, 77.93026264, -56.08619269

_BUILD_CACHE = {}


# ------------------------------------------------------------------ host prep

def _lam():
    f = np.arange(8, dtype=np.float64)
    return (100.0 / (2 * np.pi) * WAVE_LEN ** (-f / 8)).astype(np.float32)


def _ebc():
    # EBC[k, r, m] = 1 if k == 2 + r   (select cx row 2 / cy row 3 of ROWS)
    E = np.zeros((8, 2, 128), np.float32)
    E[2, 0, :] = 1.0
    E[3, 1, :] = 1.0
    return E


def _host_constants(Wg, bg, bf16):
    """Data-dependent packed weights + static phase constants (f-major)."""
    lam = _lam()
    Wg = np.asarray(Wg, np.float32)

    # nonsep blocks: (delta, sincos) -> Wg column range
    # Wg cols: sin: dx 0-7, dy 8-15, dw 16-23, dh 24-31; cos: +32
    # partition packing is ii-major: k = ii*8 + f (matches broadcast-DMA order)
    blocks = [(0, 0), (1, 8), (2, 32), (3, 40)]  # (blk, col0): xs, ys, xc, yc
    WgPk = np.zeros((4, 128, 128), np.float32)
    for blk, col0 in blocks:
        for h in range(H):
            for i16 in range(16):
                m = 16 * h + i16
                for fi in range(8):
                    k = i16 * 8 + fi           # ii-major
                    WgPk[blk, k, m] = Wg[h, col0 + fi]

    # sep: A_h (32, 32) over U = [sin aw(8), cos aw(8), sin ah(8), cos ah(8)]
    A = np.zeros((H, 32, 32), np.float32)
    for h in range(H):
        wsw, wcw = Wg[h, 16:24], Wg[h, 48:56]
        wsh, wch = Wg[h, 24:32], Wg[h, 56:64]
        for fi in range(8):
            A[h, fi, 8 + fi] += wsw[fi]
            A[h, 8 + fi, fi] += -wsw[fi]
            A[h, 8 + fi, 8 + fi] += wcw[fi]
            A[h, fi, fi] += wcw[fi]
            A[h, 16 + fi, 24 + fi] += wsh[fi]
            A[h, 24 + fi, 16 + fi] += -wsh[fi]
            A[h, 24 + fi, 24 + fi] += wch[fi]
            A[h, 16 + fi, 16 + fi] += wch[fi]
    Ask = A.transpose(1, 0, 2)  # (32 k, 8 h, 32 f'): Ask[k, h, f'] = A[h, k, f']

    LAMV = np.tile(lam, 16)[:, None].astype(np.float32)        # (128, 1) ii-major
    LAM232 = np.zeros((2, 32), np.float32)                     # U outer lhsT
    LAM232[0, 0:8] = lam; LAM232[0, 8:16] = lam                # dw rows (sin, cos)
    LAM232[1, 16:24] = lam; LAM232[1, 24:32] = lam             # dh rows
    SHIFT32 = np.zeros((32, 1), np.float32)
    SHIFT32[8:16] = 0.25; SHIFT32[24:32] = 0.25                # cos rows

    return {
        "WgPk": WgPk.astype(bf16),
        "Ask": np.ascontiguousarray(Ask).astype(bf16),
        "LAMV": LAMV,
        "LAM232": LAM232,
        "SHIFT32": SHIFT32,
        "bg_row": np.asarray(bg, np.float32).reshape(1, 8),
        "EBC": _ebc(),
    }


# ------------------------------------------------------------- custom DVE ops

def _register_op(name, spec_builder):
    from concourse import dve_ops
    from concourse.dve_uop import DveOpSpec

    for o in dve_ops.OPS:
        if o.name == name:
            return o
    spec = spec_builder()
    shas = {}
    for ver in ("v3", "v4"):
        try:
            from concourse.dve_spec import lower
            s = DveOpSpec(name=name, opcode=0, uops=lower(spec, ver=ver),
                          rd1_en=_has_src1(spec))
            shas[ver] = s.sha(ver)
        except Exception:
            pass
    op = dve_ops.DveOp(name, spec, subdim=False, uops_sha=shas)
    dve_ops.OPS.append(op)
    dve_ops.CUSTOM_DVE_SPECS[name] = spec
    dve_ops._SUB_OPCODE_FOR_NAME[name] = max(dve_ops._SUB_OPCODE_FOR_NAME.values()) + 1
    return op


def _has_src1(spec):
    from concourse.dve_ops import has_src1
    return has_src1(spec)


def _register_frac():
    from concourse.dve_spec import Spec, Src0, C0, C1, C2

    def build():
        u = Src0 * C0 + C1

        def _ref(in0, in1, s0, s1, imm2):
            uu = np.float32(in0 * s0 + s1)
            k = np.float32(uu + np.float32(imm2)) - np.float32(imm2)
            return np.float32(uu - k)

        return Spec(body=u - ((u + C2) - C2), reference=_ref)

    return _register_op("FRAC0", build)


def _register_sinpoly():
    from concourse.dve_spec import Spec, Src0, C0, C1, C2, C3, _spill_c3_to_src1

    def build():
        r2 = Src0 * Src0
        body = Src0 * (C0 + r2 * (C1 + r2 * (C2 + r2 * C3)))

        def _ref(in0, in1, s0, s1, imm2):
            r2 = np.float32(in0 * in0)
            h = np.float32(s0) + r2 * (np.float32(s1)
                                       + r2 * (np.float32(imm2) + r2 * np.float32(in1)))
            return np.float32(in0 * h)

        return Spec(body=_spill_c3_to_src1(body), reference=_ref)

    return _register_op("SINPOLY7", build)


def _register_absclip():
    from concourse.dve_spec import Spec, Src0, C0, C1, C2, Zero, maxx

    def build():
        t = (Src0 - C0) * C1

        def _ref(in0, in1, s0, s1, imm2):
            t = np.float32((in0 - s0) * s1)
            return np.float32(np.maximum(np.abs(t), np.float32(imm2)))

        return Spec(body=maxx(maxx(t, Zero - t), C2), reference=_ref)

    return _register_op("ABSCLIP", build)


# ---------------------------------------------------------------- the kernel

def _build_nc(debug=False):
    import concourse.bass as bass
    import concourse.mybir as mybir
    from concourse import tile, masks, bacc

    dt = mybir.dt
    AF = mybir.ActivationFunctionType
    ALU = mybir.AluOpType
    FRAC = _register_frac()
    SINP = _register_sinpoly()
    ACLP = _register_absclip()

    nc = bacc.Bacc("TRN2", target_bir_lowering=False, debug=False)
    P = lambda n, s, io: nc.dram_tensor(
        n, s, dt.float32, kind="ExternalOutput" if io else "ExternalInput").ap()
    Pb = lambda n, s: nc.dram_tensor(n, s, dt.bfloat16, kind="ExternalInput").ap()

    x_d = Pb("x2b", [BL, N, D])
    boxes_d = P("boxes2", [BL, N, 4], False)
    Wq_d, Wk_d, Wv_d, Wo_d = (Pb(n, [D, D]) for n in ("Wqb", "Wkb", "Wvb", "Wob"))
    bqs_d = P("bqs", [D], False)
    bk_d = P("bkv", [D], False)
    bo_d = P("bov", [D], False)
    WgPk_d = Pb("WgPk", [4, 128, 128])
    Ask_d = Pb("Ask", [32, H, 32])
    LAMV_d = P("LAMV", [128, 1], False)
    LAM232_d = P("LAM232", [2, 32], False)
    SHIFT32_d = P("SHIFT32", [32, 1], False)
    bg_d = P("bg_row", [1, H], False)
    bvb_d = Pb("bvb", [128, D])
    EBC_d = P("EBC", [8, 2, 128], False)
    out_d = P("out2", [BL, N, D], True)
    if debug:
        Db = lambda n, s: nc.dram_tensor(n, s, dt.bfloat16, kind="ExternalOutput").ap()
        dbg_lnd = nc.dram_tensor("dbg_lnd", [128, BL, 2, 2, N], dt.float32, kind="ExternalOutput").ap()
        dbg_rhs = Db("dbg_rhs", [128, 4, N])   # b0 gi=0 sin/cos tiles
        dbg_gsb = Db("dbg_gsb", [128, N])
        dbg_V33 = Db("dbg_V33", [33, BL, N])
        dbg_qT = Db("dbg_qT", [128, H, 2 * N])
        dbg_kT = Db("dbg_kT", [128, H, 2 * N])
        dbg_v = Db("dbg_v", [128, BL, 2, D])
        dbg_outT = Db("dbg_outT", [128, H, BL, N])

    f32, bf16 = dt.float32, dt.bfloat16

    with tile.TileContext(nc) as tc, ExitStack() as ctx:
        pool = ctx.enter_context(tc.tile_pool(name="resident", bufs=1))
        wk = ctx.enter_context(tc.tile_pool(name="work", bufs=2))
        wks = ctx.enter_context(tc.tile_pool(name="works", bufs=2))
        wkb = ctx.enter_context(tc.tile_pool(name="workb", bufs=3))
        ps_pv = ctx.enter_context(tc.tile_pool(name="ps_pv", bufs=2, space="PSUM"))
        ps_eg = ctx.enter_context(tc.tile_pool(name="ps_eg", bufs=2, space="PSUM"))
        ps_gt = ctx.enter_context(tc.tile_pool(name="ps_gt", bufs=2, space="PSUM"))
        ps_acc = ctx.enter_context(tc.tile_pool(name="ps_acc", bufs=2, space="PSUM"))

        # ---------- resident loads / constants
        Wq_sb = pool.tile([128, 8, D], bf16, tag="wqy")
        Wk_sb = pool.tile([128, 8, D], bf16, tag="wko")
        Wv_sb = pool.tile([128, 8, D], bf16)
        Wo_sb = pool.tile([128, 8, D], bf16)
        WgPk_sb = pool.tile([128, 4, 128], bf16)
        nc.sync.dma_start(WgPk_sb[:], WgPk_d.rearrange("b p m -> p b m"))
        Ask_sb = pool.tile([32, H, 32], bf16); nc.sync.dma_start(Ask_sb[:], Ask_d[:])
        LAMV_sb = pool.tile([128, 1], f32); nc.sync.dma_start(LAMV_sb[:], LAMV_d[:])
        LAM232_sb = pool.tile([2, 32], f32); nc.sync.dma_start(LAM232_sb[:], LAM232_d[:])
        SHIFT32_sb = pool.tile([32, 1], f32); nc.sync.dma_start(SHIFT32_sb[:], SHIFT32_d[:])
        bg_sb = pool.tile([1, H], f32); nc.sync.dma_start(bg_sb[:], bg_d[:])
        bq_c = pool.tile([128, 8], f32); nc.sync.dma_start(bq_c[:], bqs_d.rearrange("(t p) -> p t", p=128))
        bk_c = pool.tile([128, 8], f32); nc.sync.dma_start(bk_c[:], bk_d.rearrange("(t p) -> p t", p=128))
        bo_c = pool.tile([128, 8], f32); nc.sync.dma_start(bo_c[:], bo_d.rearrange("(t p) -> p t", p=128))

        id_bf = pool.tile([128, 128], bf16)
        masks.make_identity(nc, id_bf[:])
        id_f32 = pool.tile([128, 128], f32)
        masks.make_identity(nc, id_f32[:])

        ONESBF = pool.tile([128, 128], bf16); nc.vector.memset(ONESBF[:], 1.0)
        ZEROBF = pool.tile([128, 128], bf16); nc.vector.memset(ZEROBF[:], 0.0)
        C3T = pool.tile([128, 1], f32); nc.vector.memset(C3T[:], SC3)
        bvb = pool.tile([128, D], bf16); nc.sync.dma_start(bvb[:], bvb_d[:])
        EBC_sb = pool.tile([8, 2, 128], f32); nc.sync.dma_start(EBC_sb[:], EBC_d[:])

        gAT = pool.tile([128, BL, 2, H, N], bf16)   # (j, b, jh, h, i) relu'd geo^T
        xT = pool.tile([128, 8, 2 * N], bf16)

        # ========== PHASE A: boxes prep (Ln region), both batches ==========
        # one fused fp32 delta tile for both batches -> ONE Ln instruction
        da_all = pool.tile([128, BL, 2, 2, N], f32)   # (p, b, it, d, j)
        lnd_all = pool.tile([128, BL, 2, 2, N], f32)
        rows_b, cols_b = {}, {}
        for b in range(BL):
            bx = wk.tile([128, 2, 4], f32, tag="bx")
            nc.sync.dma_start(bx[:], boxes_d[b].rearrange("(tt p) c -> p tt c", p=128))
            cols = wk.tile([128, 2, 8], f32, tag="cols")  # lnw lnh cx cy rw rh w h
            cols_b[b] = cols
            for tt in range(2):
                c = cols[:, tt, :]
                nc.vector.scalar_tensor_tensor(c[:, 6:7], bx[:, tt, 2:3], 1.0, bx[:, tt, 0:1], ALU.add, ALU.subtract)
                nc.vector.scalar_tensor_tensor(c[:, 7:8], bx[:, tt, 3:4], 1.0, bx[:, tt, 1:2], ALU.add, ALU.subtract)
                nc.vector.scalar_tensor_tensor(c[:, 2:3], bx[:, tt, 0:1], 1.0, bx[:, tt, 2:3], ALU.mult, ALU.add)
                nc.vector.tensor_scalar(c[:, 2:3], c[:, 2:3], 0.5, None, ALU.mult)
                nc.vector.scalar_tensor_tensor(c[:, 3:4], bx[:, tt, 1:2], 1.0, bx[:, tt, 3:4], ALU.mult, ALU.add)
                nc.vector.tensor_scalar(c[:, 3:4], c[:, 3:4], 0.5, None, ALU.mult)
                nc.vector.reciprocal(c[:, 4:5], c[:, 6:7])
                nc.vector.reciprocal(c[:, 5:6], c[:, 7:8])
                nc.scalar.activation(c[:, 0:2], c[:, 6:8], AF.Ln)

            rows = wk.tile([8, N], f32, tag="rows")
            rows_b[b] = rows
            for tt in range(2):
                rp = ps_acc.tile([8, 128], f32, tag="acc")
                nc.tensor.transpose(rp[:], cols[:, tt, :], id_f32[:])
                nc.scalar.copy(rows[:, bass.ts(tt, 128)], rp[:])

            cb = wk.tile([128, 2, N], f32, tag="cb")
            for r in range(2):
                bp = ps_acc.tile([128, N], f32, tag="acc")
                nc.tensor.matmul(bp[:], EBC_sb[:, r, :], rows[:], start=True, stop=True)
                nc.scalar.copy(cb[:, r, :], bp[:])

            for it in range(2):
                for d in range(2):
                    nc.vector._custom_dve(
                        ACLP, out=da_all[:, b, it, d, :], in0=cb[:, d, :],
                        s0=cols[:, it, 2 + d:3 + d], s1=cols[:, it, 4 + d:5 + d],
                        imm2=1e-3)
        # the single Ln: every geo Sin transitively depends on this
        nc.scalar.activation(lnd_all[:], da_all[:], AF.Ln)
        if debug:
            nc.sync.dma_start(dbg_lnd[:], lnd_all[:])

        # ========== PHASE B: V33 / PU (DVE sin), both batches ==========
        V33_b, PU_b = {}, {}
        for b in range(BL):
            rows = rows_b[b]
            V33 = wk.tile([33, N], bf16, tag="V33")
            V33_b[b] = V33
            up = ps_acc.tile([32, N], f32, tag="acc")
            nc.tensor.matmul(up[:], LAM232_sb[:], rows[0:2, :], start=True, stop=True)
            ur = wks.tile([32, N], f32, tag="ur")
            nc.vector._custom_dve(FRAC, out=ur[:], in0=up[:], s0=1.0, s1=SHIFT32_sb[:], imm2=C_ROUND)
            nc.vector._custom_dve(SINP, out=V33[0:32, :], in0=ur[:], in1=C3T[0:32, :],
                                  s0=SC0, s1=SC1, imm2=SC2)
            nc.vector.memset(V33[32:33, :], 1.0)
            if debug:
                nc.sync.dma_start(dbg_V33[:, b, :], V33[:])

            PU = wk.tile([33, 16, 128], bf16, tag="PU")
            PU_b[b] = PU
            for h in range(H):
                pp = ps_acc.tile([32, N], f32, tag="acc")
                nc.tensor.matmul(pp[:], Ask_sb[:, h, :], V33[0:32, :], start=True, stop=True)
                nc.scalar.copy(PU[0:32, :, 16 * h:16 * h + 16], pp[:].rearrange("p (g i) -> p g i", g=16))
                nc.vector.tensor_scalar(PU[32:33, :, 16 * h:16 * h + 16],
                                        V33[32:33, :].rearrange("p (g i) -> p g i", g=16),
                                        bg_sb[0:1, h:h + 1], None, ALU.mult)

        # x transposed loads (bf16, DMA xbar transpose), then weight DMAs,
        # so the tiny phase-A DMAs still go first
        for b in range(BL):
            for kt in range(8):
                nc.sync.dma_start_transpose(
                    xT[:, kt, b * N:(b + 1) * N], x_d[b][:, bass.ts(kt, 128)])
        nc.sync.dma_start(Wq_sb[:], Wq_d.rearrange("(kt p) n -> p kt n", p=128))
        nc.sync.dma_start(Wk_sb[:], Wk_d.rearrange("(kt p) n -> p kt n", p=128))
        nc.sync.dma_start(Wv_sb[:], Wv_d.rearrange("(kt p) n -> p kt n", p=128))
        nc.sync.dma_start(Wo_sb[:], Wo_d.rearrange("(kt p) n -> p kt n", p=128))

        qT = pool.tile([128, H, 2 * N], bf16)
        kT = pool.tile([128, H, 2 * N], bf16)
        v_sb = pool.tile([128, BL, 2, D], bf16)

        def q_unit(b, mt):
            qps = ps_pv.tile([128, N], f32, tag="pv")
            for kt in range(8):
                nc.tensor.matmul(qps[:], Wq_sb[:, kt, bass.ts(mt, 128)],
                                 xT[:, kt, b * N:(b + 1) * N],
                                 start=(kt == 0), stop=(kt == 7))
            nc.scalar.activation(qT[:, mt, b * N:(b + 1) * N], qps[:], AF.Identity,
                                 bias=bq_c[:, mt:mt + 1], scale=1.0)

        def k_unit(b, mt):
            kps = ps_pv.tile([128, N], f32, tag="pv")
            for kt in range(8):
                nc.tensor.matmul(kps[:], Wk_sb[:, kt, bass.ts(mt, 128)],
                                 xT[:, kt, b * N:(b + 1) * N],
                                 start=(kt == 0), stop=(kt == 7))
            nc.vector.tensor_scalar(kT[:, mt, b * N:(b + 1) * N], kps[:],
                                    bk_c[:, mt:mt + 1], None, ALU.add)

        def v_unit(b, u):
            tt, chk = divmod(u, 2)
            vps = ps_pv.tile([128, 512], f32, tag="pv")
            for kt in range(8):
                nc.tensor.matmul(vps[:], xT[:, kt, b * N + tt * 128:b * N + (tt + 1) * 128],
                                 Wv_sb[:, kt, bass.ts(chk, 512)],
                                 start=(kt == 0), stop=(kt == 7))
            nc.vector.scalar_tensor_tensor(
                v_sb[:, b, tt, bass.ts(chk, 512)], vps[:], 1.0,
                bvb[:, bass.ts(chk, 512)], ALU.mult, ALU.add)

        # ========== geo gi-loop, QKV issue-interleaved ==========
        gsb_last = [None]
        for b in range(BL):
            V33, PU = V33_b[b], PU_b[b]
            units = ([lambda b=b, mt=mt: q_unit(b, mt) for mt in range(8)]
                     + [lambda b=b, mt=mt: k_unit(b, mt) for mt in range(8)]
                     + [lambda b=b, u=u: v_unit(b, u) for u in range(4)])
            for gi in range(16):
                it, gsub = divmod(gi, 8)
                # broadcast-DMA expansion: 16 source rows -> 128 (f-major)
                rin = wkb.tile([128, 2, N], f32, tag="rin")
                src = lnd_all[16 * gsub:16 * gsub + 16, b, it, :, :]
                nc.sync.dma_start(
                    rin[:], src.unsqueeze(1).broadcast_to((16, 8, 2, N)))
                rr4 = wkb.tile([128, 4, N], f32, tag="rr4")
                for sc in range(2):
                    nc.vector._custom_dve(FRAC, out=rr4[:, 2 * sc:2 * sc + 2, :], in0=rin[:],
                                          s0=LAMV_sb[:], s1=0.25 * sc, imm2=C_ROUND)
                rhs = wkb.tile([128, 4, N], bf16, tag="rhs")   # (p, blk, j)
                nc.scalar.activation(rhs[:], rr4[:], AF.Sin, bias=0.0, scale=TWO_PI)
                if debug and b == 0 and gi == 0:
                    nc.sync.dma_start(dbg_rhs[:], rhs[:])
                gps = ps_eg.tile([128, N], f32, tag="eg")
                for blk in range(4):
                    nc.tensor.matmul(gps[:], WgPk_sb[:, blk, :], rhs[:, blk, :],
                                     start=(blk == 0), stop=False)
                nc.tensor.matmul(gps[:], PU[:, gi, :], V33[:],
                                 start=False, stop=True)
                gsb = wks.tile([128, N], bf16, tag="gsb")
                nc.vector.tensor_scalar(gsb[:], gps[:], 0.0, None, ALU.max)
                gsb_last[0] = gsb
                if debug and b == 0 and gi == 0:
                    nc.sync.dma_start(dbg_gsb[:], gsb[:])
                gt2 = ps_gt.tile([128, 2, 128], bf16, tag="gt")
                for jh in range(2):
                    nc.tensor.transpose(gt2[:, jh, :], gsb[:, bass.ts(jh, 128)], id_bf[:])
                    if jh == 0:
                        nc.vector.tensor_copy(gAT[:, b, jh, :, bass.ts(gi, 16)],
                                              gt2[:, jh, :].rearrange("p (h i) -> p h i", h=8))
                    else:
                        nc.scalar.copy(gAT[:, b, jh, :, bass.ts(gi, 16)],
                                       gt2[:, jh, :].rearrange("p (h i) -> p h i", h=8))
                # interleave QKV work for this batch
                units[gi]()
                if gi < 4:
                    units[16 + gi]()

        if debug:
            nc.sync.dma_start(dbg_qT[:], qT[:])
            nc.sync.dma_start(dbg_kT[:], kT[:])
            nc.sync.dma_start(dbg_v[:], v_sb[:])

        # ========== attention + O-projection, per batch ==========
        y_all = pool.tile([128, 4, D], f32, tag="wqy")   # reuses Wq slot (dead)
        outT = pool.tile([128, H, BL, N], bf16, tag="wko")   # reuses Wk slot
        for b in range(BL):
            for h in range(H):
                otp = ps_acc.tile([128, N], f32, tag="acc")
                dnb = ps_acc.tile([128, N], f32, tag="acc")
                for jh in range(2):
                    stp = ps_gt.tile([128, N], f32, tag="gt")
                    nc.tensor.matmul(stp[:],
                                     kT[:, h, b * N + jh * 128:b * N + (jh + 1) * 128],
                                     qT[:, h, b * N:(b + 1) * N], start=True, stop=False)
                    # zero-weight guard: gates Exp behind the last geo tile so
                    # the ACT table never switches back from Exp to Sin
                    nc.tensor.matmul(stp[:], ZEROBF[:], gsb_last[0][:, 0:N],
                                     start=False, stop=True)
                    pt = wks.tile([128, N], bf16, tag="pt")
                    nc.scalar.activation(pt[:], stp[:], AF.Exp)
                    un = wks.tile([128, N], bf16, tag="un")
                    nc.gpsimd.tensor_mul(un[:], pt[:], gAT[:, b, jh, h, :])
                    nc.tensor.matmul(dnb[:], ONESBF[:], un[:],
                                     start=(jh == 0), stop=(jh == 1))
                    nc.tensor.matmul(otp[:], v_sb[:, b, jh, bass.ts(h, 128)],
                                     un[:], start=(jh == 0), stop=(jh == 1))
                rcb = wks.tile([128, N], f32, tag="rcb")
                nc.vector.reciprocal(rcb[:], dnb[:])
                nc.vector.tensor_mul(outT[:, h, b, :], otp[:], rcb[:])

            if debug:
                for hh in range(H):
                    nc.sync.dma_start(dbg_outT[:, hh, b, :], outT[:, hh, b, :])
            # O-projection for this batch
            for mt in range(8):
                yps = ps_eg.tile([128, N], f32, tag="eg")
                for h in range(8):
                    nc.tensor.matmul(yps[:], Wo_sb[:, h, bass.ts(mt, 128)],
                                     outT[:, h, b, :], start=(h == 0), stop=(h == 7))
                ysb = wk.tile([128, N], f32, tag="ysb")
                nc.vector.tensor_scalar(ysb[:], yps[:], bo_c[:, mt:mt + 1], None, ALU.add)
                yt2 = ps_eg.tile([128, 2, 128], f32, tag="eg")
                for tt in range(2):
                    nc.tensor.transpose(yt2[:, tt, :], ysb[:, bass.ts(tt, 128)], id_f32[:])
                nc.vector.tensor_copy(y_all[:, b * 2:b * 2 + 2, bass.ts(mt, 128)], yt2[:])
            for tt in range(2):
                nc.sync.dma_start(out_d[b, tt * 128:(tt + 1) * 128, :],
                                  y_all[:, b * 2 + tt, :])

    nc.compile()
    return nc


def _get_nc():
    if "nc" not in _BUILD_CACHE:
        _BUILD_CACHE["nc"] = _build_nc()
    return _BUILD_CACHE["nc"]


def _make_in_maps(inputs):
    import concourse.mybir as mybir

    bf16 = mybir.dt.np(mybir.dt.bfloat16)
    x = np.asarray(inputs["x"], np.float32)
    boxes = np.asarray(inputs["boxes"], np.float32)
    consts = _host_constants(inputs["Wg"], inputs["bg"], bf16)
    shared = {
        "Wqb": (np.asarray(inputs["Wq"], np.float32) * INV_SQRT_DK).astype(bf16),
        "Wkb": np.asarray(inputs["Wk"], np.float32).astype(bf16),
        "Wvb": np.asarray(inputs["Wv"], np.float32).astype(bf16),
        "Wob": np.asarray(inputs["Wo"], np.float32).astype(bf16),
        "bqs": (np.asarray(inputs["bq"], np.float32) * INV_SQRT_DK),
        "bkv": np.asarray(inputs["bk"], np.float32),
        "bov": np.asarray(inputs["bo"], np.float32),
        "bvb": np.tile(np.asarray(inputs["bv"], np.float32)[None, :], (128, 1)).astype(bf16),
        **consts,
    }
    in_maps = []
    for c in range(NCORES):
        m = dict(shared)
        m["x2b"] = np.ascontiguousarray(x[c * BL:(c + 1) * BL]).astype(bf16)
        m["boxes2"] = np.ascontiguousarray(boxes[c * BL:(c + 1) * BL])
        in_maps.append(m)
    return in_maps


def kernel(**inputs):
    from concourse.bass_utils import run_bass_kernel_spmd

    nc = _get_nc()
    in_maps = _make_in_maps(inputs)
    res = run_bass_kernel_spmd(nc, in_maps, list(range(NCORES)))
    out = np.concatenate([res.results[c]["out2"] for c in range(NCORES)], axis=0)
    return out.astype(np.float32)


if __name__ == "__main__":
    import reference as ref
    inputs = {k: np.asarray(v) for k, v in ref.setup_inputs().items()}
    expected = np.asarray(ref.reference(**inputs))
    actual = kernel(**inputs)
    err = np.abs(actual - expected)
    scale = np.abs(expected).max()
    print(f"max_abs={err.max():.3e} scale={scale:.3f} rel={err.max()/scale:.3e}")
